# revision 11
# baseline (speedup 1.0000x reference)
"""Trainium2 Bass kernel for nn_AttnReadout (segment attention readout).

Computation (reference):
    anchor[b]  = mean of ifeat rows in segment b                  [B, D]
    e[i]       = sigmoid(ifeat @ Wu.T + (anchor @ Wv.T + bv)[seg]) @ we
    alpha      = segment_softmax(e)
    rst[b]     = sum_i alpha[i] * ifeat[i]                        [B, D]
    out        = concat([rst, anchor], axis=1)                    [B, 2D]

Sharding: 2048 segments -> 8 cores x 2 windows of 128 contiguous segments.
Nodes (sorted by segment) are padded per-window to T_W tiles of 128 rows.
Segment reductions are one-hot matmuls on the tensor engine (bf16 operands,
f32 PSUM accumulation); per-segment gathers are one-hot-transposed matmuls.
ifeat arrives in two host-prepared layouts: natural [node, feat] (with a
ones column for counts/denominator) and transposed [feat, node] for the
fc_u projection. exp() is computed as sigmoid(e)/sigmoid(-e) so the scalar
engine never switches activation tables.
"""

import numpy as np
import ml_dtypes

N = 102400
D = 256
B = 2048
N_CORES = 8
W_PER_CORE = 2
N_WINDOWS = N_CORES * W_PER_CORE  # 16
SEGS_PER_WINDOW = B // N_WINDOWS  # 128
P = 128
BF = ml_dtypes.bfloat16


def _apply_tile_patch():
    """Split TileContext's multi-wait tail drain into single-wait drains
    (this walrus build rejects >1 sync wait on a Drain instruction)."""
    import concourse.tile as tile_mod
    from concourse.vector_clock import ScopedClock

    if getattr(tile_mod.TileContext, "_drain_wait_split_patch", False):
        return

    def _patched(self, tick_clock, wait_clock):
        nc = self.nc
        drain_inst = nc.sync.drain()
        wait_clock.add_sem_waits(
            drain_inst.ins, ScopedClock({None: tick_clock.global_clock})
        )
        si = drain_inst.ins.sync_info
        waits = list(si.on_wait)
        if len(waits) > 1:
            SyncInfo = type(si)
            drain_inst.ins.sync_info = SyncInfo(
                on_wait=[waits[0]], on_update=list(si.on_update)
            )
            for w in waits[1:]:
                extra = nc.sync.drain()
                extra.ins.sync_info = SyncInfo(on_wait=[w], on_update=[])

        nc.all_engine_barrier()
        assert self.sems is not None
        popped = nc._tile_sem_poison_stack.pop()
        assert popped is self._sem_poison
        nc.clear_and_free_semaphores(list(self.sems.allocated().values()))
        nc.all_engine_barrier()

    tile_mod.TileContext._drain_and_barrier = _patched
    tile_mod.TileContext._drain_wait_split_patch = True


def _split_sync_waits(nc, limit=1):
    """Split >limit sync waits per instruction into preceding single-wait
    EventSemaphore carriers on the same engine (walrus build limit)."""
    import concourse.mybir as mybir

    n_new = 0
    for _, bassbb in nc.bb_map.items():
        insts = bassbb.bb.instructions  # live list
        snapshot = list(insts)
        offset = 0
        for pos, inst in enumerate(snapshot):
            si = getattr(inst, "sync_info", None)
            if si is None:
                continue
            waits = list(si.on_wait)
            if len(waits) <= limit:
                continue
            SyncInfo = type(si)
            inst.sync_info = SyncInfo(
                on_wait=waits[:limit], on_update=list(si.on_update))
            carriers = []
            for w in waits[limit:]:
                c = mybir.InstEventSemaphore(
                    name=f"WSPLIT-{nc.next_id()}", ins=[], outs=[])
                c.engine = inst.engine
                c.sync_info = SyncInfo(on_wait=[w], on_update=[])
                carriers.append(c)
            insts[pos + offset:pos + offset] = carriers
            offset += len(carriers)
            n_new += len(carriers)
    return n_new


def _build(T_W, repeat=1):
    """Build the single-core SPMD Bass program; T_W must be even."""
    import contextlib
    import concourse.bass as bass
    import concourse.mybir as mybir
    from concourse.tile import TileContext

    _apply_tile_patch()

    f32 = mybir.dt.float32
    bf16 = mybir.dt.bfloat16
    Alu = mybir.AluOpType
    Act = mybir.ActivationFunctionType

    assert T_W % 2 == 0
    CH = T_W // 2          # tiles per DMA chunk (2 chunks per window)
    NT = W_PER_CORE * T_W

    nc = bass.Bass("TRN2", num_devices=N_CORES)

    nat_dram = nc.dram_tensor("natp", [P, NT, D + 1], bf16, kind="ExternalInput")
    ifT_dram = nc.dram_tensor("iftp", [P, NT, 2, P], bf16, kind="ExternalInput")
    seg_dram = nc.dram_tensor("segp", [P, NT], f32, kind="ExternalInput")
    wuT_dram = nc.dram_tensor("wuT", [2, P, D], bf16, kind="ExternalInput")
    wvT_dram = nc.dram_tensor("wvT", [2, P, D], bf16, kind="ExternalInput")
    web_dram = nc.dram_tensor("web", [P, D], bf16, kind="ExternalInput")
    bvb_dram = nc.dram_tensor("bvb", [P, D], f32, kind="ExternalInput")
    idf_dram = nc.dram_tensor("idf", [P, P], f32, kind="ExternalInput")
    idb_dram = nc.dram_tensor("idb", [P, P], bf16, kind="ExternalInput")
    iota_dram = nc.dram_tensor("iota", [P, P], bf16, kind="ExternalInput")
    out_dram = nc.dram_tensor("out", [W_PER_CORE, P, 2 * D], f32,
                              kind="ExternalOutput")

    with TileContext(nc) as tc:
        with contextlib.ExitStack() as ctx:
            const_pool = ctx.enter_context(tc.tile_pool(name="const", bufs=1))
            data_pool = ctx.enter_context(tc.tile_pool(name="data", bufs=1))
            ohw_pool = ctx.enter_context(tc.tile_pool(name="ohw", bufs=2))
            ohT_pool = ctx.enter_context(tc.tile_pool(name="ohT", bufs=2))
            s_pool = ctx.enter_context(tc.tile_pool(name="s", bufs=2))
            prod_pool = ctx.enter_context(tc.tile_pool(name="prod", bufs=2))
            zx_pool = ctx.enter_context(tc.tile_pool(name="zx", bufs=2))
            col_pool = ctx.enter_context(tc.tile_pool(name="col", bufs=6))
            wnd_pool = ctx.enter_context(tc.tile_pool(name="wnd", bufs=2))
            anchor_ps_pool = ctx.enter_context(
                tc.tile_pool(name="anchor_ps", bufs=1, space="PSUM"))
            wsum_ps_pool = ctx.enter_context(
                tc.tile_pool(name="wsum_ps", bufs=1, space="PSUM"))
            trb_ps_pool = ctx.enter_context(
                tc.tile_pool(name="trb_ps", bufs=2, space="PSUM"))
            trf_ps_pool = ctx.enter_context(
                tc.tile_pool(name="trf_ps", bufs=1, space="PSUM"))
            s_ps_pool = ctx.enter_context(
                tc.tile_pool(name="s_ps", bufs=2, space="PSUM"))
            fv_ps_pool = ctx.enter_context(
                tc.tile_pool(name="fv_ps", bufs=1, space="PSUM"))

            # constants
            wuT_sb = const_pool.tile([P, 2, D], bf16, name="wuT_sb", tag="wuT_sb")
            nc.sync.dma_start(wuT_sb[:], wuT_dram[:].rearrange("k p d -> p k d"))
            wvT_sb = const_pool.tile([P, 2, D], bf16, name="wvT_sb", tag="wvT_sb")
            nc.sync.dma_start(wvT_sb[:], wvT_dram[:].rearrange("k p d -> p k d"))
            web_sb = const_pool.tile([P, D], bf16, name="web_sb", tag="web_sb")
            nc.sync.dma_start(web_sb[:], web_dram[:])
            bvb_sb = const_pool.tile([P, D], f32, name="bvb_sb", tag="bvb_sb")
            nc.sync.dma_start(bvb_sb[:], bvb_dram[:])
            idf_sb = const_pool.tile([P, P], f32, name="idf_sb", tag="idf_sb")
            nc.sync.dma_start(idf_sb[:], idf_dram[:])
            idb_sb = const_pool.tile([P, P], bf16, name="idb_sb", tag="idb_sb")
            nc.sync.dma_start(idb_sb[:], idb_dram[:])
            iota_sb = const_pool.tile([P, P], bf16, name="iota_sb", tag="iota_sb")
            nc.sync.dma_start(iota_sb[:], iota_dram[:])
            seg_sb = const_pool.tile([P, NT], f32, name="seg_sb", tag="seg_sb")
            nc.sync.dma_start(seg_sb[:], seg_dram[:])

            # node data, chunked loads (2 chunks per window)
            nat_ch = []
            ifT_ch = []
            for c in range(2 * W_PER_CORE):
                natc = data_pool.tile([P, CH, D + 1], bf16, name=f"natc{c}",
                                      tag=f"natc{c}")
                nc.sync.dma_start(natc[:], nat_dram[:, c * CH:(c + 1) * CH, :])
                nat_ch.append(natc)
                iftc = data_pool.tile([P, CH, 2, P], bf16, name=f"iftc{c}",
                                      tag=f"iftc{c}")
                nc.sync.dma_start(iftc[:], ifT_dram[:, c * CH:(c + 1) * CH, :, :])
                ifT_ch.append(iftc)

            def nat_t(g):
                return nat_ch[g // CH][:, g % CH, :]

            def ifT_t(g, kb):
                return ifT_ch[g // CH][:, g % CH, kb, :]

            for rep in range(repeat):
              for w in range(W_PER_CORE):
                # ---- pass 1: one-hots + anchor (segment mean) ----
                ohw = ohw_pool.tile([P, T_W, P], bf16, name=f"ohw{rep}_{w}",
                                    tag="ohw")
                anchor_ps = anchor_ps_pool.tile([P, D + 1], f32,
                                                name=f"anc_ps{rep}_{w}",
                                                tag="anchor_ps")
                for t in range(T_W):
                    g = w * T_W + t
                    nc.vector.tensor_scalar(
                        ohw[:, t, :], iota_sb[:], seg_sb[:, g:g + 1], None,
                        Alu.is_equal)
                    nc.tensor.matmul(anchor_ps[:], ohw[:, t, :], nat_t(g),
                                     start=(t == 0), stop=(t == T_W - 1))
                cnt = col_pool.tile([P, 1], f32, name=f"cnt{rep}_{w}", tag="col")
                nc.vector.tensor_scalar(cnt[:], anchor_ps[:, D:D + 1], 1.0,
                                        None, Alu.max)
                rcnt = col_pool.tile([P, 1], f32, name=f"rcnt{rep}_{w}", tag="col")
                nc.vector.reciprocal(rcnt[:], cnt[:])
                out_sb = wnd_pool.tile([P, 2 * D], f32, name=f"osb{rep}_{w}",
                                       tag="out_sb")
                nc.vector.tensor_scalar(out_sb[:, D:2 * D], anchor_ps[:, 0:D],
                                        rcnt[:], None, Alu.mult)

                # feat_v = anchor @ Wv.T + bv  (via transposed anchor)
                anchT = wnd_pool.tile([P, 2, P], bf16, name=f"anchT{rep}_{w}",
                                      tag="anchT")
                for db in range(2):
                    trf = trf_ps_pool.tile([P, P], f32, name=f"atr{rep}_{w}{db}",
                                           tag="trf")
                    nc.tensor.transpose(trf[:], out_sb[:, D + db * P:D + (db + 1) * P],
                                        idf_sb[:])
                    nc.scalar.copy(anchT[:, db, :], trf[:])
                fv_ps = fv_ps_pool.tile([P, D], f32, name=f"fv_ps{rep}_{w}",
                                        tag="fv_ps")
                for db in range(2):
                    nc.tensor.matmul(fv_ps[:], anchT[:, db, :], wvT_sb[:, db, :],
                                     start=(db == 0), stop=(db == 1))
                fv_sb = wnd_pool.tile([P, D], bf16, name=f"fv{rep}_{w}", tag="fv")
                nc.vector.tensor_tensor(fv_sb[:], fv_ps[:], bvb_sb[:], Alu.add)

                # ---- pass 2: logits, segment softmax, weighted sum ----
                wsum_ps = wsum_ps_pool.tile([P, D + 1], f32,
                                            name=f"wsum_ps{rep}_{w}",
                                            tag="wsum_ps")
                for t in range(T_W):
                    g = w * T_W + t
                    trb = trb_ps_pool.tile([P, P], bf16, name=f"ohTp{rep}_{g}",
                                           tag="trb")
                    nc.tensor.transpose(trb[:], ohw[:, t, :], idb_sb[:])
                    ohT = ohT_pool.tile([P, P], bf16, name=f"ohT{rep}_{g}",
                                        tag="ohT")
                    nc.scalar.copy(ohT[:], trb[:])

                    s_ps = s_ps_pool.tile([P, D], f32, name=f"s_ps{rep}_{g}",
                                          tag="s_ps")
                    nc.tensor.matmul(s_ps[:], ifT_t(g, 0), wuT_sb[:, 0, :],
                                     start=True, stop=False)
                    nc.tensor.matmul(s_ps[:], ifT_t(g, 1), wuT_sb[:, 1, :],
                                     start=False, stop=False)
                    nc.tensor.matmul(s_ps[:], ohT[:], fv_sb[:],
                                     start=False, stop=True)
                    s_sb = s_pool.tile([P, D], bf16, name=f"s{rep}_{g}", tag="s")
                    nc.scalar.activation(s_sb[:], s_ps[:], Act.Sigmoid)
                    prod = prod_pool.tile([P, D], bf16, name=f"pr{rep}_{g}",
                                          tag="prod")
                    e_col = col_pool.tile([P, 1], f32, name=f"e{rep}_{g}",
                                          tag="col")
                    nc.vector.scalar_tensor_tensor(
                        out=prod[:], in0=s_sb[:], scalar=1.0, in1=web_sb[:],
                        op0=Alu.mult, op1=Alu.mult, accum_out=e_col[:])
                    # z = exp(e) = sigmoid(e) / sigmoid(-e): stays on the
                    # sigmoid ACT table (a sigmoid<->exp table swap costs
                    # ~1.3us per switch).
                    sp_col = col_pool.tile([P, 1], f32, name=f"sp{rep}_{g}",
                                           tag="col")
                    nc.scalar.activation(sp_col[:], e_col[:], Act.Sigmoid)
                    sn_col = col_pool.tile([P, 1], f32, name=f"sn{rep}_{g}",
                                           tag="col")
                    nc.scalar.activation(sn_col[:], e_col[:], Act.Sigmoid,
                                         scale=-1.0)
                    rn_col = col_pool.tile([P, 1], f32, name=f"rn{rep}_{g}",
                                           tag="col")
                    nc.vector.reciprocal(rn_col[:], sn_col[:])
                    z_col = col_pool.tile([P, 1], f32, name=f"z{rep}_{g}",
                                          tag="col")
                    nc.vector.tensor_tensor(z_col[:], sp_col[:], rn_col[:],
                                            Alu.mult)
                    zx = zx_pool.tile([P, D + 1], bf16, name=f"zx{rep}_{g}",
                                      tag="zx")
                    nc.vector.tensor_scalar(zx[:], nat_t(g), z_col[:], None,
                                            Alu.mult)
                    nc.tensor.matmul(wsum_ps[:], ohw[:, t, :], zx[:],
                                     start=(t == 0), stop=(t == T_W - 1))
                den = col_pool.tile([P, 1], f32, name=f"den{rep}_{w}", tag="col")
                nc.vector.tensor_scalar(den[:], wsum_ps[:, D:D + 1], 1e-30,
                                        None, Alu.max)
                rden = col_pool.tile([P, 1], f32, name=f"rden{rep}_{w}",
                                     tag="col")
                nc.vector.reciprocal(rden[:], den[:])
                nc.vector.tensor_scalar(out_sb[:, 0:D], wsum_ps[:, 0:D],
                                        rden[:], None, Alu.mult)
                nc.sync.dma_start(out_dram[w], out_sb[:])

    return nc


def _prepare(ifeat, Wu, Wv, bv, we, seg_ids):
    """Host-side shard + pad + layout. Returns (T_W, in_maps)."""
    ifeat = np.asarray(ifeat, dtype=np.float32)
    Wu = np.asarray(Wu, dtype=np.float32)
    Wv = np.asarray(Wv, dtype=np.float32)
    bv = np.asarray(bv, dtype=np.float32)
    we = np.asarray(we, dtype=np.float32)
    seg_ids = np.asarray(seg_ids)

    bounds = np.searchsorted(
        seg_ids, np.arange(0, B + 1, SEGS_PER_WINDOW), side="left")
    n_w = np.diff(bounds)
    T_W = max(2, int(-(-int(n_w.max()) // P)))
    if T_W % 2:
        T_W += 1
    NT = W_PER_CORE * T_W

    wuT = np.ascontiguousarray(Wu.T).reshape(2, P, D).astype(BF)
    wvT = np.ascontiguousarray(Wv.T).reshape(2, P, D).astype(BF)
    web = np.tile(we, (P, 1)).astype(BF)
    bvb = np.tile(bv, (P, 1)).astype(np.float32)
    idf = np.eye(P, dtype=np.float32)
    idb = np.eye(P, dtype=BF)
    iota = np.tile(np.arange(P, dtype=np.float32), (P, 1)).astype(BF)

    in_maps = []
    for c in range(N_CORES):
        nat = np.zeros((NT * P, D + 1), dtype=np.float32)
        nat[:, D] = 1.0
        seg = np.full((NT * P,), 500.0, dtype=np.float32)
        for wl in range(W_PER_CORE):
            w = c * W_PER_CORE + wl
            lo, hi = bounds[w], bounds[w + 1]
            base = wl * T_W * P
            nat[base:base + (hi - lo), 0:D] = ifeat[lo:hi]
            seg[base:base + (hi - lo)] = (
                seg_ids[lo:hi].astype(np.float32) - w * SEGS_PER_WINDOW)
        natb = nat.astype(BF).reshape(NT, P, D + 1)
        # partition-major layouts
        natp = np.ascontiguousarray(natb.transpose(1, 0, 2))      # [P, NT, 257]
        x = nat[:, 0:D].astype(BF).reshape(NT, P, 2, P)           # [g,i,kb,d]
        iftp = np.ascontiguousarray(x.transpose(3, 0, 2, 1))      # [d, g, kb, i]
        segp = np.ascontiguousarray(seg.reshape(NT, P).T)         # [P, NT]
        in_maps.append({
            "natp": natp, "iftp": iftp, "segp": segp,
            "wuT": wuT, "wvT": wvT, "web": web, "bvb": bvb,
            "idf": idf, "idb": idb, "iota": iota,
        })
    return T_W, in_maps


_LAST = {}


def _run(ifeat, Wu, Wv, bv, we, seg_ids, trace=False):
    from concourse.bass_utils import run_bass_kernel_spmd

    T_W, in_maps = _prepare(ifeat, Wu, Wv, bv, we, seg_ids)
    nc = _build(T_W)
    _split_sync_waits(nc)
    res = run_bass_kernel_spmd(nc, in_maps, list(range(N_CORES)), trace=trace)
    _LAST["res"] = res
    _LAST["T_W"] = T_W
    _LAST["nc"] = nc
    _LAST["in_maps"] = in_maps

    out = np.empty((B, 2 * D), dtype=np.float32)
    for c in range(N_CORES):
        core_out = res.results[c]["out"]  # [W_PER_CORE, P, 2D]
        for wl in range(W_PER_CORE):
            w = c * W_PER_CORE + wl
            out[w * SEGS_PER_WINDOW:(w + 1) * SEGS_PER_WINDOW, :] = core_out[wl]
    return out


def kernel(ifeat, Wu, Wv, bv, we, seg_ids):
    return _run(ifeat, Wu, Wv, bv, we, seg_ids, trace=False)


# revision 19
# speedup vs baseline: 41.9086x; 41.9086x over previous
"""Trainium2 Bass kernel for nn_AttnReadout (segment attention readout).

Computation (reference):
    anchor[b]  = mean of ifeat rows in segment b                  [B, D]
    e[i]       = sigmoid(ifeat @ Wu.T + (anchor @ Wv.T + bv)[seg]) @ we
    alpha      = segment_softmax(e)
    rst[b]     = sum_i alpha[i] * ifeat[i]                        [B, D]
    out        = concat([rst, anchor], axis=1)                    [B, 2D]

Sharding: 2048 segments -> 8 cores x 2 windows of 128 contiguous segments.
Nodes (sorted by segment) are padded per-window to T_W tiles of 128 rows.
Segment reductions are one-hot matmuls on the tensor engine (bf16 operands,
f32 PSUM accumulation); per-segment gathers are one-hot-transposed matmuls.
ifeat arrives in two host-prepared layouts: natural [node, feat] (with a
ones column for counts/denominator) and transposed [feat, node] for the
fc_u projection. exp() is batched to one activation per window so the
scalar engine switches between the sigmoid and exp tables only twice per
window (a table swap costs ~1.3us).
"""

import numpy as np
import ml_dtypes

N = 102400
D = 256
B = 2048
N_CORES = 8
W_PER_CORE = 2
N_WINDOWS = N_CORES * W_PER_CORE  # 16
SEGS_PER_WINDOW = B // N_WINDOWS  # 128
P = 128
BF = ml_dtypes.bfloat16


def _apply_tile_patch():
    """Split TileContext's multi-wait tail drain into single-wait drains
    (this walrus build rejects >1 sync wait on a Drain instruction)."""
    import concourse.tile as tile_mod
    from concourse.vector_clock import ScopedClock

    if getattr(tile_mod.TileContext, "_drain_wait_split_patch", False):
        return

    def _patched(self, tick_clock, wait_clock):
        nc = self.nc
        drain_inst = nc.sync.drain()
        wait_clock.add_sem_waits(
            drain_inst.ins, ScopedClock({None: tick_clock.global_clock})
        )
        si = drain_inst.ins.sync_info
        waits = list(si.on_wait) if si is not None else []
        if len(waits) > 1:
            SyncInfo = type(si)
            drain_inst.ins.sync_info = SyncInfo(
                on_wait=[waits[0]], on_update=list(si.on_update)
            )
            for w in waits[1:]:
                extra = nc.sync.drain()
                extra.ins.sync_info = SyncInfo(on_wait=[w], on_update=[])

        nc.all_engine_barrier()
        assert self.sems is not None
        popped = nc._tile_sem_poison_stack.pop()
        assert popped is self._sem_poison
        nc.clear_and_free_semaphores(list(self.sems.allocated().values()))
        nc.all_engine_barrier()

    tile_mod.TileContext._drain_and_barrier = _patched
    tile_mod.TileContext._drain_wait_split_patch = True


def _split_sync_waits(nc, limit=1):
    """Split >limit sync waits per instruction into preceding single-wait
    EventSemaphore carriers on the same engine (walrus build limit)."""
    import concourse.mybir as mybir

    n_new = 0
    for _, bassbb in nc.bb_map.items():
        insts = bassbb.bb.instructions  # live list
        snapshot = list(insts)
        offset = 0
        for pos, inst in enumerate(snapshot):
            si = getattr(inst, "sync_info", None)
            if si is None:
                continue
            waits = list(si.on_wait)
            if len(waits) <= limit:
                continue
            SyncInfo = type(si)
            inst.sync_info = SyncInfo(
                on_wait=waits[:limit], on_update=list(si.on_update))
            carriers = []
            for w in waits[limit:]:
                c = mybir.InstEventSemaphore(
                    name=f"WSPLIT-{nc.next_id()}", ins=[], outs=[])
                c.engine = inst.engine
                c.sync_info = SyncInfo(on_wait=[w], on_update=[])
                carriers.append(c)
            insts[pos + offset:pos + offset] = carriers
            offset += len(carriers)
            n_new += len(carriers)
    return n_new


def _build(T_W, repeat=1, loop_repeat=None):
    """Build the single-core SPMD Bass program; T_W must be even."""
    import contextlib
    import concourse.bass as bass
    import concourse.mybir as mybir
    from concourse.tile import TileContext

    _apply_tile_patch()

    f32 = mybir.dt.float32
    bf16 = mybir.dt.bfloat16
    Alu = mybir.AluOpType
    Act = mybir.ActivationFunctionType

    assert T_W % 4 == 0
    CH = T_W // 4          # tiles per DMA chunk (4 chunks per window)
    NT = W_PER_CORE * T_W

    nc = bass.Bass("TRN2", num_devices=N_CORES)

    nat_dram = nc.dram_tensor("natp", [P, NT, D + 1], bf16, kind="ExternalInput")
    ifT_dram = nc.dram_tensor("iftp", [P, NT, 2, P], bf16, kind="ExternalInput")
    seg_dram = nc.dram_tensor("segp", [P, NT], f32, kind="ExternalInput")
    wuT_dram = nc.dram_tensor("wuT", [2, P, D], bf16, kind="ExternalInput")
    wvT_dram = nc.dram_tensor("wvT", [2, P, D], bf16, kind="ExternalInput")
    web_dram = nc.dram_tensor("web", [P, D], bf16, kind="ExternalInput")
    bvb_dram = nc.dram_tensor("bvb", [P, D], f32, kind="ExternalInput")
    idf_dram = nc.dram_tensor("idf", [P, P], f32, kind="ExternalInput")
    idb_dram = nc.dram_tensor("idb", [P, P], bf16, kind="ExternalInput")
    iota_dram = nc.dram_tensor("iota", [P, P], bf16, kind="ExternalInput")
    out_dram = nc.dram_tensor("out", [W_PER_CORE, P, 2 * D], f32,
                              kind="ExternalOutput")

    with TileContext(nc) as tc:
        with contextlib.ExitStack() as ctx:
            const_pool = ctx.enter_context(tc.tile_pool(name="const", bufs=1))
            data_pool = ctx.enter_context(tc.tile_pool(name="data", bufs=1))
            ohw_pool = ctx.enter_context(tc.tile_pool(name="ohw", bufs=3))
            ohT_pool = ctx.enter_context(tc.tile_pool(name="ohT", bufs=4))
            s_pool = ctx.enter_context(tc.tile_pool(name="s", bufs=4))
            prod_pool = ctx.enter_context(tc.tile_pool(name="prod", bufs=4))
            zx_pool = ctx.enter_context(tc.tile_pool(name="zx", bufs=4))
            col_pool = ctx.enter_context(tc.tile_pool(name="col", bufs=12))
            wnd_pool = ctx.enter_context(tc.tile_pool(name="wnd", bufs=4))
            anchor_ps_pool = ctx.enter_context(
                tc.tile_pool(name="anchor_ps", bufs=1, space="PSUM"))
            wsum_ps_pool = ctx.enter_context(
                tc.tile_pool(name="wsum_ps", bufs=1, space="PSUM"))
            trb_ps_pool = ctx.enter_context(
                tc.tile_pool(name="trb_ps", bufs=2, space="PSUM"))
            s_ps_pool = ctx.enter_context(
                tc.tile_pool(name="s_ps", bufs=3, space="PSUM"))
            fv_ps_pool = ctx.enter_context(
                tc.tile_pool(name="fv_ps", bufs=1, space="PSUM"))

            # constants
            wuT_sb = const_pool.tile([P, 2, D], bf16, name="wuT_sb", tag="wuT_sb")
            nc.sync.dma_start(wuT_sb[:], wuT_dram[:].rearrange("k p d -> p k d"))
            wvT_sb = const_pool.tile([P, 2, D], bf16, name="wvT_sb", tag="wvT_sb")
            nc.sync.dma_start(wvT_sb[:], wvT_dram[:].rearrange("k p d -> p k d"))
            web_sb = const_pool.tile([P, D], bf16, name="web_sb", tag="web_sb")
            nc.sync.dma_start(web_sb[:], web_dram[:])
            bvb_sb = const_pool.tile([P, D], f32, name="bvb_sb", tag="bvb_sb")
            nc.sync.dma_start(bvb_sb[:], bvb_dram[:])
            idb_sb = const_pool.tile([P, P], bf16, name="idb_sb", tag="idb_sb")
            nc.sync.dma_start(idb_sb[:], idb_dram[:])
            iota_sb = const_pool.tile([P, P], bf16, name="iota_sb", tag="iota_sb")
            nc.sync.dma_start(iota_sb[:], iota_dram[:])
            seg_sb = const_pool.tile([P, NT], f32, name="seg_sb", tag="seg_sb")
            nc.sync.dma_start(seg_sb[:], seg_dram[:])

            def body(rep):
              for w in range(W_PER_CORE):
                # window node data, chunked loads (2 chunks per window)
                nat_ch = {}
                ifT_ch = {}
                for cl in range(4):
                    c = 4 * w + cl
                    natc = data_pool.tile([P, CH, D + 1], bf16,
                                          name=f"natc{rep}_{c}", tag="natc",
                                          bufs=8)
                    nc.sync.dma_start(natc[:], nat_dram[:, c * CH:(c + 1) * CH, :])
                    nat_ch[c] = natc
                    iftc = data_pool.tile([P, CH, 2, P], bf16,
                                          name=f"iftc{rep}_{c}", tag="iftc",
                                          bufs=8)
                    nc.sync.dma_start(iftc[:], ifT_dram[:, c * CH:(c + 1) * CH, :, :])
                    ifT_ch[c] = iftc

                def nat_t(g):
                    return nat_ch[g // CH][:, g % CH, :]

                def ifT_t(g, kb):
                    return ifT_ch[g // CH][:, g % CH, kb, :]

                # ---- pass 1: one-hots + anchor (segment mean) ----
                ohw = ohw_pool.tile([P, T_W, P], bf16, name=f"ohw{rep}_{w}",
                                    tag="ohw")
                anchor_ps = anchor_ps_pool.tile([P, D + 1], f32,
                                                name=f"anc_ps{rep}_{w}",
                                                tag="anchor_ps")
                for t in range(T_W):
                    g = w * T_W + t
                    nc.vector.tensor_scalar(
                        ohw[:, t, :], iota_sb[:], seg_sb[:, g:g + 1], None,
                        Alu.is_equal)
                    nc.tensor.matmul(anchor_ps[:], ohw[:, t, :], nat_t(g),
                                     start=(t == 0), stop=(t == T_W - 1))
                cnt = col_pool.tile([P, 1], f32, name=f"cnt{rep}_{w}", tag="col")
                nc.vector.tensor_scalar(cnt[:], anchor_ps[:, D:D + 1], 1.0,
                                        None, Alu.max)
                rcnt = col_pool.tile([P, 1], f32, name=f"rcnt{rep}_{w}", tag="col")
                nc.vector.reciprocal(rcnt[:], cnt[:])
                out_sb = wnd_pool.tile([P, 2 * D], f32, name=f"osb{rep}_{w}",
                                       tag="out_sb")
                nc.vector.tensor_scalar(out_sb[:, D:2 * D], anchor_ps[:, 0:D],
                                        rcnt[:], None, Alu.mult)

                # feat_v = anchor @ Wv.T + bv  (via transposed anchor)
                anchor_bf = wnd_pool.tile([P, D], bf16, name=f"anbf{rep}_{w}",
                                          tag="anchor_bf")
                nc.vector.tensor_scalar(anchor_bf[:], anchor_ps[:, 0:D],
                                        rcnt[:], None, Alu.mult)
                anchT = wnd_pool.tile([P, 2, P], bf16, name=f"anchT{rep}_{w}",
                                      tag="anchT")
                for db in range(2):
                    trf = trb_ps_pool.tile([P, P], bf16, name=f"atr{rep}_{w}{db}",
                                           tag="trb")
                    nc.tensor.transpose(trf[:], anchor_bf[:, db * P:(db + 1) * P],
                                        idb_sb[:])
                    nc.any.tensor_copy(anchT[:, db, :], trf[:])
                fv_ps = fv_ps_pool.tile([P, D], f32, name=f"fv_ps{rep}_{w}",
                                        tag="fv_ps")
                for db in range(2):
                    nc.tensor.matmul(fv_ps[:], anchT[:, db, :], wvT_sb[:, db, :],
                                     start=(db == 0), stop=(db == 1))
                fv_sb = wnd_pool.tile([P, D], bf16, name=f"fv{rep}_{w}", tag="fv")
                nc.vector.tensor_tensor(fv_sb[:], fv_ps[:], bvb_sb[:], Alu.add)

                # ---- pass 2: logits, segment softmax, weighted sum ----
                wsum_ps = wsum_ps_pool.tile([P, D + 1], f32,
                                            name=f"wsum_ps{rep}_{w}",
                                            tag="wsum_ps")
                e_win = wnd_pool.tile([P, T_W], f32, name=f"ew{rep}_{w}",
                                      tag="e_win")
                for t in range(T_W):
                    g = w * T_W + t
                    trb = trb_ps_pool.tile([P, P], bf16, name=f"ohTp{rep}_{g}",
                                           tag="trb")
                    nc.tensor.transpose(trb[:], ohw[:, t, :], idb_sb[:])
                    ohT = ohT_pool.tile([P, P], bf16, name=f"ohT{rep}_{g}",
                                        tag="ohT")
                    nc.any.tensor_copy(ohT[:], trb[:])

                    s_ps = s_ps_pool.tile([P, D], f32, name=f"s_ps{rep}_{g}",
                                          tag="s_ps")
                    nc.tensor.matmul(s_ps[:], ifT_t(g, 0), wuT_sb[:, 0, :],
                                     start=True, stop=False)
                    nc.tensor.matmul(s_ps[:], ifT_t(g, 1), wuT_sb[:, 1, :],
                                     start=False, stop=False)
                    nc.tensor.matmul(s_ps[:], ohT[:], fv_sb[:],
                                     start=False, stop=True)
                    s_sb = s_pool.tile([P, D], bf16, name=f"s{rep}_{g}", tag="s")
                    nc.scalar.activation(s_sb[:], s_ps[:], Act.Sigmoid)
                    prod = prod_pool.tile([P, D], bf16, name=f"pr{rep}_{g}",
                                          tag="prod")
                    nc.vector.scalar_tensor_tensor(
                        out=prod[:], in0=s_sb[:], scalar=1.0, in1=web_sb[:],
                        op0=Alu.mult, op1=Alu.mult,
                        accum_out=e_win[:, t:t + 1])
                # one exp per window amortizes the sigmoid<->exp ACT table
                # swap (~1.3us per switch)
                z_win = wnd_pool.tile([P, T_W], f32, name=f"zw{rep}_{w}",
                                      tag="z_win")
                nc.scalar.activation(z_win[:], e_win[:], Act.Exp)
                for t in range(T_W):
                    g = w * T_W + t
                    zx = zx_pool.tile([P, D + 1], bf16, name=f"zx{rep}_{g}",
                                      tag="zx")
                    nc.vector.tensor_scalar(zx[:], nat_t(g), z_win[:, t:t + 1],
                                            None, Alu.mult)
                    nc.tensor.matmul(wsum_ps[:], ohw[:, t, :], zx[:],
                                     start=(t == 0), stop=(t == T_W - 1))
                den = col_pool.tile([P, 1], f32, name=f"den{rep}_{w}", tag="col")
                nc.vector.tensor_scalar(den[:], wsum_ps[:, D:D + 1], 1e-30,
                                        None, Alu.max)
                rden = col_pool.tile([P, 1], f32, name=f"rden{rep}_{w}",
                                     tag="col")
                nc.vector.reciprocal(rden[:], den[:])
                nc.vector.tensor_scalar(out_sb[:, 0:D], wsum_ps[:, 0:D],
                                        rden[:], None, Alu.mult)
                nc.sync.dma_start(out_dram[w], out_sb[:])

            if loop_repeat is not None:
                with tc.For_i(0, loop_repeat, 1):
                    body("L")
            else:
                for rep in range(repeat):
                    body(rep)

    return nc


def _prepare(ifeat, Wu, Wv, bv, we, seg_ids):
    """Host-side shard + pad + layout. Returns (T_W, in_maps)."""
    ifeat = np.asarray(ifeat, dtype=np.float32)
    Wu = np.asarray(Wu, dtype=np.float32)
    Wv = np.asarray(Wv, dtype=np.float32)
    bv = np.asarray(bv, dtype=np.float32)
    we = np.asarray(we, dtype=np.float32)
    seg_ids = np.asarray(seg_ids)

    bounds = np.searchsorted(
        seg_ids, np.arange(0, B + 1, SEGS_PER_WINDOW), side="left")
    n_w = np.diff(bounds)
    T_W = max(4, int(-(-int(n_w.max()) // P)))
    T_W = ((T_W + 3) // 4) * 4
    NT = W_PER_CORE * T_W

    wuT = np.ascontiguousarray(Wu.T).reshape(2, P, D).astype(BF)
    wvT = np.ascontiguousarray(Wv.T).reshape(2, P, D).astype(BF)
    web = np.tile(we, (P, 1)).astype(BF)
    bvb = np.tile(bv, (P, 1)).astype(np.float32)
    idf = np.eye(P, dtype=np.float32)
    idb = np.eye(P, dtype=BF)
    iota = np.tile(np.arange(P, dtype=np.float32), (P, 1)).astype(BF)

    in_maps = []
    for c in range(N_CORES):
        nat = np.zeros((NT * P, D + 1), dtype=np.float32)
        nat[:, D] = 1.0
        seg = np.full((NT * P,), 500.0, dtype=np.float32)
        for wl in range(W_PER_CORE):
            w = c * W_PER_CORE + wl
            lo, hi = bounds[w], bounds[w + 1]
            base = wl * T_W * P
            nat[base:base + (hi - lo), 0:D] = ifeat[lo:hi]
            seg[base:base + (hi - lo)] = (
                seg_ids[lo:hi].astype(np.float32) - w * SEGS_PER_WINDOW)
        natb = nat.astype(BF).reshape(NT, P, D + 1)
        # partition-major layouts
        natp = np.ascontiguousarray(natb.transpose(1, 0, 2))      # [P, NT, 257]
        x = nat[:, 0:D].astype(BF).reshape(NT, P, 2, P)           # [g,i,kb,d]
        iftp = np.ascontiguousarray(x.transpose(3, 0, 2, 1))      # [d, g, kb, i]
        segp = np.ascontiguousarray(seg.reshape(NT, P).T)         # [P, NT]
        in_maps.append({
            "natp": natp, "iftp": iftp, "segp": segp,
            "wuT": wuT, "wvT": wvT, "web": web, "bvb": bvb,
            "idf": idf, "idb": idb, "iota": iota,
        })
    return T_W, in_maps


_LAST = {}


def _run(ifeat, Wu, Wv, bv, we, seg_ids, trace=False):
    from concourse.bass_utils import run_bass_kernel_spmd

    T_W, in_maps = _prepare(ifeat, Wu, Wv, bv, we, seg_ids)
    nc = _build(T_W)
    _split_sync_waits(nc)
    res = run_bass_kernel_spmd(nc, in_maps, list(range(N_CORES)), trace=trace)
    _LAST["res"] = res
    _LAST["T_W"] = T_W
    _LAST["nc"] = nc
    _LAST["in_maps"] = in_maps

    out = np.empty((B, 2 * D), dtype=np.float32)
    for c in range(N_CORES):
        core_out = res.results[c]["out"]  # [W_PER_CORE, P, 2D]
        for wl in range(W_PER_CORE):
            w = c * W_PER_CORE + wl
            out[w * SEGS_PER_WINDOW:(w + 1) * SEGS_PER_WINDOW, :] = core_out[wl]
    return out


def kernel(ifeat, Wu, Wv, bv, we, seg_ids):
    return _run(ifeat, Wu, Wv, bv, we, seg_ids, trace=False)


# revision 27
# speedup vs baseline: 49.6039x; 1.1836x over previous
"""Trainium2 Bass kernel for nn_AttnReadout (segment attention readout).

Computation (reference):
    anchor[b]  = mean of ifeat rows in segment b                  [B, D]
    e[i]       = sigmoid(ifeat @ Wu.T + (anchor @ Wv.T + bv)[seg]) @ we
    alpha      = segment_softmax(e)
    rst[b]     = sum_i alpha[i] * ifeat[i]                        [B, D]
    out        = concat([rst, anchor], axis=1)                    [B, 2D]

Sharding: 2048 segments -> 8 cores x 2 windows of 128 contiguous segments.
Nodes (sorted by segment) are padded per-window to T_W tiles of 128 rows.
Segment reductions are one-hot matmuls on the tensor engine (bf16 operands,
f32 PSUM accumulation); per-segment gathers are one-hot-transposed matmuls.
ifeat arrives in two host-prepared layouts: natural [node, feat] (with a
ones column for counts/denominator) and transposed [feat, node] for the
fc_u projection. z = exp(e) is computed per 13-tile chunk as
sigmoid(e)/sigmoid(-e), which keeps the scalar engine on the sigmoid ACT
table (a sigmoid<->exp table swap costs ~1.3us) and avoids a whole-window
barrier; the weighted segment sum scales the one-hot (oh*z) rather than
the features, so the natural-layout tile is reused unscaled.
"""

import numpy as np
import ml_dtypes

N = 102400
D = 256
B = 2048
N_CORES = 8
W_PER_CORE = 2
N_WINDOWS = N_CORES * W_PER_CORE  # 16
SEGS_PER_WINDOW = B // N_WINDOWS  # 128
P = 128
BF = ml_dtypes.bfloat16


def _apply_tile_patch():
    """Split TileContext's multi-wait tail drain into single-wait drains
    (this walrus build rejects >1 sync wait on a Drain instruction)."""
    import concourse.tile as tile_mod
    from concourse.vector_clock import ScopedClock

    if getattr(tile_mod.TileContext, "_drain_wait_split_patch", False):
        return

    def _patched(self, tick_clock, wait_clock):
        nc = self.nc
        drain_inst = nc.sync.drain()
        wait_clock.add_sem_waits(
            drain_inst.ins, ScopedClock({None: tick_clock.global_clock})
        )
        si = drain_inst.ins.sync_info
        waits = list(si.on_wait) if si is not None else []
        if len(waits) > 1:
            SyncInfo = type(si)
            drain_inst.ins.sync_info = SyncInfo(
                on_wait=[waits[0]], on_update=list(si.on_update)
            )
            for w in waits[1:]:
                extra = nc.sync.drain()
                extra.ins.sync_info = SyncInfo(on_wait=[w], on_update=[])

        nc.all_engine_barrier()
        assert self.sems is not None
        popped = nc._tile_sem_poison_stack.pop()
        assert popped is self._sem_poison
        nc.clear_and_free_semaphores(list(self.sems.allocated().values()))
        nc.all_engine_barrier()

    tile_mod.TileContext._drain_and_barrier = _patched
    tile_mod.TileContext._drain_wait_split_patch = True


def _split_sync_waits(nc, limit=1):
    """Split >limit sync waits per instruction into preceding single-wait
    EventSemaphore carriers on the same engine (walrus build limit)."""
    import concourse.mybir as mybir

    n_new = 0
    for _, bassbb in nc.bb_map.items():
        insts = bassbb.bb.instructions  # live list
        snapshot = list(insts)
        offset = 0
        for pos, inst in enumerate(snapshot):
            si = getattr(inst, "sync_info", None)
            if si is None:
                continue
            waits = list(si.on_wait)
            if len(waits) <= limit:
                continue
            SyncInfo = type(si)
            inst.sync_info = SyncInfo(
                on_wait=waits[:limit], on_update=list(si.on_update))
            carriers = []
            for w in waits[limit:]:
                c = mybir.InstEventSemaphore(
                    name=f"WSPLIT-{nc.next_id()}", ins=[], outs=[])
                c.engine = inst.engine
                c.sync_info = SyncInfo(on_wait=[w], on_update=[])
                carriers.append(c)
            insts[pos + offset:pos + offset] = carriers
            offset += len(carriers)
            n_new += len(carriers)
    return n_new


def _build(T_W, repeat=1, loop_repeat=None):
    """Build the single-core SPMD Bass program; T_W must be even."""
    import contextlib
    import concourse.bass as bass
    import concourse.mybir as mybir
    from concourse.tile import TileContext

    _apply_tile_patch()

    f32 = mybir.dt.float32
    bf16 = mybir.dt.bfloat16
    Alu = mybir.AluOpType
    Act = mybir.ActivationFunctionType

    assert T_W % 4 == 0
    CH = T_W // 4          # tiles per DMA chunk (4 chunks per window)
    NT = W_PER_CORE * T_W

    nc = bass.Bass("TRN2", num_devices=N_CORES)

    nat_dram = nc.dram_tensor("natp", [P, NT, D + 1], bf16, kind="ExternalInput")
    ifT_dram = nc.dram_tensor("iftp", [P, NT, 2, P], bf16, kind="ExternalInput")
    seg_dram = nc.dram_tensor("segp", [P, NT], f32, kind="ExternalInput")
    wuT_dram = nc.dram_tensor("wuT", [2, P, D], bf16, kind="ExternalInput")
    wvT_dram = nc.dram_tensor("wvT", [2, P, D], bf16, kind="ExternalInput")
    web_dram = nc.dram_tensor("web", [P, D], bf16, kind="ExternalInput")
    bvb_dram = nc.dram_tensor("bvb", [P, D], f32, kind="ExternalInput")
    idf_dram = nc.dram_tensor("idf", [P, P], f32, kind="ExternalInput")
    idb_dram = nc.dram_tensor("idb", [P, P], bf16, kind="ExternalInput")
    iota_dram = nc.dram_tensor("iota", [P, P], bf16, kind="ExternalInput")
    out_dram = nc.dram_tensor("out", [W_PER_CORE, P, 2 * D], f32,
                              kind="ExternalOutput")

    with TileContext(nc) as tc:
        with contextlib.ExitStack() as ctx:
            const_pool = ctx.enter_context(tc.tile_pool(name="const", bufs=1))
            data_pool = ctx.enter_context(tc.tile_pool(name="data", bufs=1))
            ohw_pool = ctx.enter_context(tc.tile_pool(name="ohw", bufs=3))
            ohT_pool = ctx.enter_context(tc.tile_pool(name="ohT", bufs=4))
            s_pool = ctx.enter_context(tc.tile_pool(name="s", bufs=4))
            prod_pool = ctx.enter_context(tc.tile_pool(name="prod", bufs=4))
            zx_pool = ctx.enter_context(tc.tile_pool(name="zx", bufs=4))
            col_pool = ctx.enter_context(tc.tile_pool(name="col", bufs=12))
            zch_pool = ctx.enter_context(tc.tile_pool(name="zch", bufs=8))
            wnd_pool = ctx.enter_context(tc.tile_pool(name="wnd", bufs=4))
            anchor_ps_pool = ctx.enter_context(
                tc.tile_pool(name="anchor_ps", bufs=1, space="PSUM"))
            wsum_ps_pool = ctx.enter_context(
                tc.tile_pool(name="wsum_ps", bufs=1, space="PSUM"))
            trb_ps_pool = ctx.enter_context(
                tc.tile_pool(name="trb_ps", bufs=2, space="PSUM"))
            s_ps_pool = ctx.enter_context(
                tc.tile_pool(name="s_ps", bufs=3, space="PSUM"))
            fv_ps_pool = ctx.enter_context(
                tc.tile_pool(name="fv_ps", bufs=1, space="PSUM"))

            # constants
            wuT_sb = const_pool.tile([P, 2, D], bf16, name="wuT_sb", tag="wuT_sb")
            nc.sync.dma_start(wuT_sb[:], wuT_dram[:].rearrange("k p d -> p k d"))
            wvT_sb = const_pool.tile([P, 2, D], bf16, name="wvT_sb", tag="wvT_sb")
            nc.sync.dma_start(wvT_sb[:], wvT_dram[:].rearrange("k p d -> p k d"))
            web_sb = const_pool.tile([P, D], bf16, name="web_sb", tag="web_sb")
            nc.sync.dma_start(web_sb[:], web_dram[:])
            bvb_sb = const_pool.tile([P, D], f32, name="bvb_sb", tag="bvb_sb")
            nc.sync.dma_start(bvb_sb[:], bvb_dram[:])
            idb_sb = const_pool.tile([P, P], bf16, name="idb_sb", tag="idb_sb")
            nc.sync.dma_start(idb_sb[:], idb_dram[:])
            iota_sb = const_pool.tile([P, P], bf16, name="iota_sb", tag="iota_sb")
            nc.sync.dma_start(iota_sb[:], iota_dram[:])
            seg_sb = const_pool.tile([P, NT], f32, name="seg_sb", tag="seg_sb")
            nc.sync.dma_start(seg_sb[:], seg_dram[:])

            def body(rep):
              for w in range(W_PER_CORE):
                # window node data, chunked loads (2 chunks per window)
                nat_ch = {}
                ifT_ch = {}
                for cl in range(4):
                    c = 4 * w + cl
                    natc = data_pool.tile([P, CH, D + 1], bf16,
                                          name=f"natc{rep}_{c}", tag="natc",
                                          bufs=8)
                    nc.sync.dma_start(natc[:], nat_dram[:, c * CH:(c + 1) * CH, :])
                    nat_ch[c] = natc
                    iftc = data_pool.tile([P, CH, 2, P], bf16,
                                          name=f"iftc{rep}_{c}", tag="iftc",
                                          bufs=8)
                    nc.sync.dma_start(iftc[:], ifT_dram[:, c * CH:(c + 1) * CH, :, :])
                    ifT_ch[c] = iftc

                def nat_t(g):
                    return nat_ch[g // CH][:, g % CH, :]

                def ifT_t(g, kb):
                    return ifT_ch[g // CH][:, g % CH, kb, :]

                # ---- pass 1: one-hots + anchor (segment mean) ----
                ohw = ohw_pool.tile([P, T_W, P], bf16, name=f"ohw{rep}_{w}",
                                    tag="ohw")
                anchor_ps = anchor_ps_pool.tile([P, D + 1], f32,
                                                name=f"anc_ps{rep}_{w}",
                                                tag="anchor_ps")
                for t in range(T_W):
                    g = w * T_W + t
                    nc.vector.tensor_scalar(
                        ohw[:, t, :], iota_sb[:], seg_sb[:, g:g + 1], None,
                        Alu.is_equal)
                    nc.tensor.matmul(anchor_ps[:], ohw[:, t, :], nat_t(g),
                                     start=(t == 0), stop=(t == T_W - 1))
                cnt = col_pool.tile([P, 1], f32, name=f"cnt{rep}_{w}", tag="col")
                nc.vector.tensor_scalar(cnt[:], anchor_ps[:, D:D + 1], 1.0,
                                        None, Alu.max)
                rcnt = col_pool.tile([P, 1], f32, name=f"rcnt{rep}_{w}", tag="col")
                nc.vector.reciprocal(rcnt[:], cnt[:])
                out_sb = wnd_pool.tile([P, 2 * D], f32, name=f"osb{rep}_{w}",
                                       tag="out_sb")
                nc.vector.tensor_scalar(out_sb[:, D:2 * D], anchor_ps[:, 0:D],
                                        rcnt[:], None, Alu.mult)

                # feat_v = anchor @ Wv.T + bv  (via transposed anchor)
                anchor_bf = wnd_pool.tile([P, D], bf16, name=f"anbf{rep}_{w}",
                                          tag="anchor_bf")
                nc.vector.tensor_scalar(anchor_bf[:], anchor_ps[:, 0:D],
                                        rcnt[:], None, Alu.mult)
                anchT = wnd_pool.tile([P, 2, P], bf16, name=f"anchT{rep}_{w}",
                                      tag="anchT")
                for db in range(2):
                    trf = trb_ps_pool.tile([P, P], bf16, name=f"atr{rep}_{w}{db}",
                                           tag="trb")
                    nc.tensor.transpose(trf[:], anchor_bf[:, db * P:(db + 1) * P],
                                        idb_sb[:])
                    nc.any.tensor_copy(anchT[:, db, :], trf[:])
                fv_ps = fv_ps_pool.tile([P, D], f32, name=f"fv_ps{rep}_{w}",
                                        tag="fv_ps")
                for db in range(2):
                    nc.tensor.matmul(fv_ps[:], anchT[:, db, :], wvT_sb[:, db, :],
                                     start=(db == 0), stop=(db == 1))
                fv_sb = wnd_pool.tile([P, D], bf16, name=f"fv{rep}_{w}", tag="fv")
                nc.vector.tensor_tensor(fv_sb[:], fv_ps[:], bvb_sb[:], Alu.add)

                # ---- pass 2: logits, segment softmax, weighted sum ----
                wsum_ps = wsum_ps_pool.tile([P, D + 1], f32,
                                            name=f"wsum_ps{rep}_{w}",
                                            tag="wsum_ps")
                e_win = wnd_pool.tile([P, T_W], f32, name=f"ew{rep}_{w}",
                                      tag="e_win")
                z_win = wnd_pool.tile([P, T_W], f32, name=f"zw{rep}_{w}",
                                      tag="z_win")
                for t in range(T_W):
                    g = w * T_W + t
                    trb = trb_ps_pool.tile([P, P], bf16, name=f"ohTp{rep}_{g}",
                                           tag="trb")
                    nc.tensor.transpose(trb[:], ohw[:, t, :], idb_sb[:])
                    ohT = ohT_pool.tile([P, P], bf16, name=f"ohT{rep}_{g}",
                                        tag="ohT")
                    nc.any.tensor_copy(ohT[:], trb[:])

                    s_ps = s_ps_pool.tile([P, D], f32, name=f"s_ps{rep}_{g}",
                                          tag="s_ps")
                    nc.tensor.matmul(s_ps[:], ifT_t(g, 0), wuT_sb[:, 0, :],
                                     start=True, stop=False)
                    nc.tensor.matmul(s_ps[:], ifT_t(g, 1), wuT_sb[:, 1, :],
                                     start=False, stop=False)
                    nc.tensor.matmul(s_ps[:], ohT[:], fv_sb[:],
                                     start=False, stop=True)
                    s_sb = s_pool.tile([P, D], bf16, name=f"s{rep}_{g}", tag="s")
                    nc.scalar.activation(s_sb[:], s_ps[:], Act.Sigmoid)
                    prod = prod_pool.tile([P, D], bf16, name=f"pr{rep}_{g}",
                                          tag="prod")
                    nc.vector.scalar_tensor_tensor(
                        out=prod[:], in0=s_sb[:], scalar=1.0, in1=web_sb[:],
                        op0=Alu.mult, op1=Alu.mult,
                        accum_out=e_win[:, t:t + 1])
                    if (t + 1) % CH == 0:
                        # z = exp(e) = sigmoid(e)/sigmoid(-e) per chunk: stays
                        # on the sigmoid ACT table (no table swaps) and avoids
                        # a whole-window barrier before the weighted pass.
                        c0 = t + 1 - CH
                        sp = zch_pool.tile([P, CH], f32,
                                           name=f"sp{rep}_{w}_{t}", tag="zch")
                        nc.scalar.activation(sp[:], e_win[:, c0:t + 1],
                                             Act.Sigmoid)
                        sn = zch_pool.tile([P, CH], f32,
                                           name=f"sn{rep}_{w}_{t}", tag="zch")
                        nc.scalar.activation(sn[:], e_win[:, c0:t + 1],
                                             Act.Sigmoid, scale=-1.0)
                        rn = zch_pool.tile([P, CH], f32,
                                           name=f"rn{rep}_{w}_{t}", tag="zch")
                        nc.vector.reciprocal(rn[:], sn[:])
                        nc.vector.tensor_tensor(z_win[:, c0:t + 1], sp[:],
                                                rn[:], Alu.mult)
                for t in range(T_W):
                    g = w * T_W + t
                    ohz = zx_pool.tile([P, P], bf16, name=f"ohz{rep}_{g}",
                                       tag="zx")
                    nc.vector.tensor_scalar(ohz[:], ohw[:, t, :],
                                            z_win[:, t:t + 1], None, Alu.mult)
                    nc.tensor.matmul(wsum_ps[:], ohz[:], nat_t(g),
                                     start=(t == 0), stop=(t == T_W - 1))
                den = col_pool.tile([P, 1], f32, name=f"den{rep}_{w}", tag="col")
                nc.vector.tensor_scalar(den[:], wsum_ps[:, D:D + 1], 1e-30,
                                        None, Alu.max)
                rden = col_pool.tile([P, 1], f32, name=f"rden{rep}_{w}",
                                     tag="col")
                nc.vector.reciprocal(rden[:], den[:])
                nc.vector.tensor_scalar(out_sb[:, 0:D], wsum_ps[:, 0:D],
                                        rden[:], None, Alu.mult)
                nc.sync.dma_start(out_dram[w], out_sb[:])

            if loop_repeat is not None:
                with tc.For_i(0, loop_repeat, 1):
                    body("L")
            else:
                for rep in range(repeat):
                    body(rep)

    return nc


def _prepare(ifeat, Wu, Wv, bv, we, seg_ids):
    """Host-side shard + pad + layout. Returns (T_W, in_maps)."""
    ifeat = np.asarray(ifeat, dtype=np.float32)
    Wu = np.asarray(Wu, dtype=np.float32)
    Wv = np.asarray(Wv, dtype=np.float32)
    bv = np.asarray(bv, dtype=np.float32)
    we = np.asarray(we, dtype=np.float32)
    seg_ids = np.asarray(seg_ids)

    bounds = np.searchsorted(
        seg_ids, np.arange(0, B + 1, SEGS_PER_WINDOW), side="left")
    n_w = np.diff(bounds)
    T_W = max(4, int(-(-int(n_w.max()) // P)))
    T_W = ((T_W + 3) // 4) * 4
    NT = W_PER_CORE * T_W

    wuT = np.ascontiguousarray(Wu.T).reshape(2, P, D).astype(BF)
    wvT = np.ascontiguousarray(Wv.T).reshape(2, P, D).astype(BF)
    web = np.tile(we, (P, 1)).astype(BF)
    bvb = np.tile(bv, (P, 1)).astype(np.float32)
    idf = np.eye(P, dtype=np.float32)
    idb = np.eye(P, dtype=BF)
    iota = np.tile(np.arange(P, dtype=np.float32), (P, 1)).astype(BF)

    in_maps = []
    for c in range(N_CORES):
        nat = np.zeros((NT * P, D + 1), dtype=np.float32)
        nat[:, D] = 1.0
        seg = np.full((NT * P,), 500.0, dtype=np.float32)
        for wl in range(W_PER_CORE):
            w = c * W_PER_CORE + wl
            lo, hi = bounds[w], bounds[w + 1]
            base = wl * T_W * P
            nat[base:base + (hi - lo), 0:D] = ifeat[lo:hi]
            seg[base:base + (hi - lo)] = (
                seg_ids[lo:hi].astype(np.float32) - w * SEGS_PER_WINDOW)
        natb = nat.astype(BF).reshape(NT, P, D + 1)
        # partition-major layouts
        natp = np.ascontiguousarray(natb.transpose(1, 0, 2))      # [P, NT, 257]
        x = nat[:, 0:D].astype(BF).reshape(NT, P, 2, P)           # [g,i,kb,d]
        iftp = np.ascontiguousarray(x.transpose(3, 0, 2, 1))      # [d, g, kb, i]
        segp = np.ascontiguousarray(seg.reshape(NT, P).T)         # [P, NT]
        in_maps.append({
            "natp": natp, "iftp": iftp, "segp": segp,
            "wuT": wuT, "wvT": wvT, "web": web, "bvb": bvb,
            "idf": idf, "idb": idb, "iota": iota,
        })
    return T_W, in_maps


_LAST = {}


def _run(ifeat, Wu, Wv, bv, we, seg_ids, trace=False):
    from concourse.bass_utils import run_bass_kernel_spmd

    T_W, in_maps = _prepare(ifeat, Wu, Wv, bv, we, seg_ids)
    nc = _build(T_W)
    _split_sync_waits(nc)
    res = run_bass_kernel_spmd(nc, in_maps, list(range(N_CORES)), trace=trace)
    _LAST["res"] = res
    _LAST["T_W"] = T_W
    _LAST["nc"] = nc
    _LAST["in_maps"] = in_maps

    out = np.empty((B, 2 * D), dtype=np.float32)
    for c in range(N_CORES):
        core_out = res.results[c]["out"]  # [W_PER_CORE, P, 2D]
        for wl in range(W_PER_CORE):
            w = c * W_PER_CORE + wl
            out[w * SEGS_PER_WINDOW:(w + 1) * SEGS_PER_WINDOW, :] = core_out[wl]
    return out


def kernel(ifeat, Wu, Wv, bv, we, seg_ids):
    return _run(ifeat, Wu, Wv, bv, we, seg_ids, trace=False)
